# revision 36
# baseline (speedup 1.0000x reference)
"""Trainium2 Bass kernel for nn_MultiHeadAttention_57337813402001.

B=4, S=2048, D=1024, H=16 heads (DH=64). 8 NeuronCores.

Sharding: core = (batch b, head-group hg); hg splits the 16 heads into two
groups of 8 (tensor parallel on the QKV projection output columns and the
output projection input rows), b is data parallel. Each core computes a
partial output projection for its 8 heads; the host sums the two partials
per batch and adds the (algebraically folded) bias terms.

Algebraic simplifications (exact in real arithmetic):
  - bk drops out of softmax (adds a per-query constant to scores).
  - bv commutes through the attention average: folded into a host-side bias
    row bv @ Wo^T added at the end.
  - softmax without max-subtraction: |scores|/sqrt(d) < ~0.7 here.

v2 dataflow (vs the 477us baseline):
  - Q/K path in fp8e4: the projections use MatmulPerfMode.DoubleRow (two
    d_in k-tiles per matmul -> half the instructions at the same ns/row =
    the real fp8 2x). QK^T runs plain fp8 in the natural [pair-dh, s]
    layout (DoubleRow cannot help a 64-deep contraction and its doubled
    weight load measured slower). Scores only feed exp(s/32), so fp8's ~4%
    score error is ~0.7% on probabilities; measured 3.8e-3 end to end.
  - V path and everything after exp stays fp16 (fp8 there costs ~4% output
    error).
  - Software-pipelined schedule: per (j-block, head) window, the PE emits
    [PV(step-1) | filler matmuls | QK(step)+exp(step)] per score group --
    the dependency-blocking QK last so the in-order PE queue never idles
    (the p-state model halves the PE clock for 3us after any gap). Filler
    work (V projection, later K/Q projection chunks, prior row's output
    projection) comes from a deadline + uniform-rate scheduler. The
    softmax-normalize reciprocal chain runs on DVE at window end, and its
    ones-broadcast matmul + aT multiply are deferred one window so the PE
    never waits on DVE.
  - PSUM: qk scores 2x[128,2,512] + mm 3x[128,512] + ones 1x[128,512]
    = 8 banks exactly.
"""

import os
import sys

import numpy as np

for _p in ("/opt/trn_rl_repo",):
    if _p not in sys.path and os.path.isdir(_p):
        sys.path.insert(0, _p)

B, S, D, H = 4, 2048, 1024, 16
DH = D // H          # 64
HL = H // 2          # 8 heads per core
DL = HL * DH         # 512 local hidden
P = 128
KC = D // P          # 8 d_in chunks
CC = DL // P         # 4 local d_out chunks
N_CORES = 8

QK_FP8 = True        # DoubleRow fp8 for QK^T scores
PROJ_FP8 = True      # DoubleRow fp8 for the Q/K projections (host fp8 in)


def build_bass(s=S):
    import concourse.bass as bass  # noqa: F401
    import concourse.mybir as mybir
    from concourse import bacc
    from concourse.tile import TileContext

    dt16 = mybir.dt.float16
    f8 = mybir.dt.float8e4
    f32 = mybir.dt.float32
    AF = mybir.ActivationFunctionType
    DR = mybir.MatmulPerfMode.DoubleRow

    nsk = s // P                 # sk chunks (16)
    sqb = min(512, s)            # sq block
    nsqb = s // sqb              # 4 j rows
    sb_blk = min(512, s)
    nsb = s // sb_blk            # 4 projection s blocks
    qdt = f8 if PROJ_FP8 else dt16

    nc = bacc.Bacc()
    QT = nc.declare_dram_parameter("QT", [D, s], qdt, isOutput=False)
    KT = nc.declare_dram_parameter("KT", [D, s], qdt, isOutput=False)
    VT = nc.declare_dram_parameter("VT", [D, s], dt16, isOutput=False)
    WQT = nc.declare_dram_parameter("WQT", [D, DL], qdt, isOutput=False)
    WKT = nc.declare_dram_parameter("WKT", [D, DL], qdt, isOutput=False)
    WVT = nc.declare_dram_parameter("WVT", [D, DL], dt16, isOutput=False)
    WOT = nc.declare_dram_parameter("WOT", [DL, D], dt16, isOutput=False)
    BQ = nc.declare_dram_parameter("BQ", [P, CC], f32, isOutput=False)
    OUT = nc.declare_dram_parameter("OUT", [s, D], dt16, isOutput=True)

    with TileContext(nc) as tc:
        with (
            tc.tile_pool(name="w", bufs=1) as wp,
            tc.tile_pool(name="stq", bufs=1) as stq,
            tc.tile_pool(name="stk", bufs=1) as stk,
            tc.tile_pool(name="stv", bufs=2) as stv,
            tc.tile_pool(name="qkv", bufs=1) as qkvp,
            tc.tile_pool(name="E", bufs=3) as ep,
            tc.tile_pool(name="rc", bufs=2) as rcp,
            tc.tile_pool(name="ost", bufs=3) as ostp,
            tc.tile_pool(name="qkps", bufs=2, space="PSUM") as qkps,
            tc.tile_pool(name="mmps", bufs=3, space="PSUM") as mmps,
            tc.tile_pool(name="onps", bufs=1, space="PSUM") as onps,
        ):
            # --- weights / constants ---
            wq = wp.tile([P, KC, DL], qdt, tag="wq")
            wk = wp.tile([P, KC, DL], qdt, tag="wk")
            wv = wp.tile([P, KC, DL], dt16, tag="wv")
            wo = wp.tile([P, CC, D], dt16, tag="wo")
            bq = wp.tile([P, CC], f32, tag="bq")
            ones_row = wp.tile([1, DH], dt16, tag="ones")
            qst = stq.tile([P, KC, s], qdt, tag="qst")
            kst = stk.tile([P, KC, s], qdt, tag="kst")

            # DMA issue order feeds the critical path: k projection of
            # s-block b needs only wk + kst block b; q j0 needs wq + qst
            # block 0. Everything else follows.
            def stage_blk(dst, src, blk):
                nc.sync.dma_start(
                    dst[:, :, blk * sb_blk:(blk + 1) * sb_blk],
                    src[:, blk * sb_blk:(blk + 1) * sb_blk].rearrange(
                        "(kc p) ss -> p kc ss", p=P
                    ),
                )

            nc.sync.dma_start(wk, WKT[:].rearrange("(kc p) m -> p kc m", p=P))
            for blk in range(nsb):
                stage_blk(kst, KT, blk)
            nc.sync.dma_start(wq, WQT[:].rearrange("(kc p) m -> p kc m", p=P))
            nc.sync.dma_start(bq, BQ[:])
            stage_blk(qst, QT, 0)
            nc.sync.dma_start(wv, WVT[:].rearrange("(kc p) m -> p kc m", p=P))
            for blk in range(1, nsb):
                stage_blk(qst, QT, blk)
            nc.sync.dma_start(wo, WOT[:].rearrange("(cc p) m -> p cc m", p=P))
            nc.vector.memset(ones_row, 1.0)

            # q/k fp8 in natural projection layout: [128, pair, s], partition
            # = 64*hi + dh for head 2*pair + hi. QK^T runs plain fp8 (K=64):
            # DoubleRow can't help a 64-deep contraction, and its doubled
            # weight load measured ~50ns/matmul slower.
            qT8 = qkvp.tile([P, CC, s], f8, tag="qT8")
            kT8 = qkvp.tile([P, CC, s], f8, tag="kT8")
            vpad = qkvp.tile([P, nsk, HL, DH + 1], dt16, tag="vpad")
            aT = qkvp.tile([P, CC, s], dt16, tag="aT")
            nc.vector.memset(vpad[:, :, :, DH], 1.0)

            scale = 1.0 / np.sqrt(np.float32(D)).item()

            # ---------- emission helpers ----------
            def proj_qk_chunk(xst, w, dst8, c, blk, bias=None):
                """One [128,512] chunk of a q/k projection (fp8 DoubleRow,
                2 d_in k-tiles per matmul), cast straight into dst8[:, c]."""
                ps = mmps.tile([P, sb_blk], f32, tag="mm")
                if PROJ_FP8:
                    for hf in range(2):
                        for k2 in range(KC // 2):
                            bs = blk * sb_blk + hf * 256
                            nc.tensor.matmul(
                                ps[:, hf * 256:(hf + 1) * 256],
                                lhsT=w[:, 2 * k2:2 * k2 + 2, c * P:(c + 1) * P],
                                rhs=xst[:, 2 * k2:2 * k2 + 2, bs:bs + 256],
                                start=(k2 == 0),
                                stop=(k2 == KC // 2 - 1),
                                perf_mode=DR,
                            )
                else:
                    for k in range(KC):
                        nc.tensor.matmul(
                            ps,
                            lhsT=w[:, k, c * P:(c + 1) * P],
                            rhs=xst[:, k, blk * sb_blk:(blk + 1) * sb_blk],
                            start=(k == 0),
                            stop=(k == KC - 1),
                        )
                dsl = dst8[:, c, blk * sb_blk:(blk + 1) * sb_blk]
                with nc.allow_low_precision(reason="fp8 q/k by design"):
                    if bias is not None:
                        nc.vector.tensor_scalar_add(
                            out=dsl, in0=ps, scalar1=bias[:, c:c + 1],
                        )
                    else:
                        nc.vector.tensor_copy(out=dsl, in_=ps)

            def v_chunk(xv, cg, li, gi):
                """V projection for pair-group cg (pairs 2cg,2cg+1), local
                chunk li of the staged block = global sk chunk gi; N=256."""
                ps = mmps.tile([P, 256], f32, tag="mm")
                for k in range(KC):
                    nc.tensor.matmul(
                        ps,
                        lhsT=xv[:, k, li * P:(li + 1) * P],
                        rhs=wv[:, k, cg * 256:(cg + 1) * 256],
                        start=(k == 0),
                        stop=(k == KC - 1),
                    )
                with nc.allow_low_precision(reason="fp16 v by design"):
                    nc.vector.tensor_copy(
                        out=vpad[:, gi, 4 * cg:4 * cg + 4, 0:DH],
                        in_=ps.rearrange("p (h d) -> p h d", d=DH),
                    )

            def qk_group(h, j, g, qkt):
                """Scores^T chunks 2g,2g+1 for head h, q block j -> qkt.

                N=256 halves: measured ~1.12 cy/row vs ~1.25 at N=512, and
                the two halves share one weight load."""
                p_i, hi = h // 2, h % 2
                po = 64 * hi
                for u in range(2):
                    i = 2 * g + u
                    for hf in range(2):
                        qs = j * sqb + hf * 256
                        nc.tensor.matmul(
                            qkt[:, u, hf * 256:(hf + 1) * 256],
                            lhsT=kT8[po:po + DH, p_i, i * P:(i + 1) * P],
                            rhs=qT8[po:po + DH, p_i, qs:qs + 256],
                            start=True, stop=True,
                        )

            def exp_group(E_t, g, qkt):
                with nc.allow_low_precision(reason="fp16 probs by design"):
                    nc.scalar.activation(
                        out=E_t[:, 2 * g:2 * g + 2, :], in_=qkt,
                        func=AF.Exp, scale=scale,
                    )

            # ---------- filler queue (PE work interleaved into the
            # attention pipeline; ~each item <= ~1us of PE time) ----------
            # V staging tiles cycle; stage block DMAs are emitted lazily
            # right before first use (SP queue is in-order).
            def stage_v(blk):
                xv = stv.tile([P, KC, sb_blk], dt16, tag="stv")
                nc.sync.dma_start(
                    xv,
                    VT[:, blk * sb_blk:(blk + 1) * sb_blk].rearrange(
                        "(kc p) ss -> p kc ss", p=P
                    ),
                )
                return xv

            def oproj_chunk(sc, db):
                ps = mmps.tile([P, 512], f32, tag="mm")
                for hf in range(2):
                    for c in range(CC):
                        nc.tensor.matmul(
                            ps[:, hf * 256:(hf + 1) * 256],
                            lhsT=aT[:, c, sc * P:(sc + 1) * P],
                            rhs=wo[:, c,
                                   db * 512 + hf * 256:db * 512 + hf * 256 + 256],
                            start=(c == 0),
                            stop=(c == CC - 1),
                        )
                ot = ostp.tile([P, 512], dt16, tag="ost")
                with nc.allow_low_precision(reason="fp16 partial"):
                    nc.vector.tensor_copy(out=ot, in_=ps)
                nc.sync.dma_start(
                    OUT[sc * P:(sc + 1) * P, db * 512:(db + 1) * 512], ot
                )

            # ---------- preamble: k c0 (full S) + q j0 c0 gate QK(0,0) ----
            for blk in range(nsb):
                proj_qk_chunk(kst, wk, kT8, 0, blk)
            proj_qk_chunk(qst, wq, qT8, 0, 0, bias=bq)

            # ---------- filler scheduler ----------
            # Items = (deadline_slot, release_slot, est_pe_ns, fn), emitted
            # into group slots at a uniform PE-time rate with deadline
            # forcing, so the in-order PE queue always has ready work (the
            # p-state model halves the PE clock for 3us after any idle gap).
            # slot = window * 8 + group; windows = nsteps + 1.
            steps = [(j, h) for j in range(nsqb) for h in range(HL)]
            nsteps = len(steps)
            total_slots = (nsteps + 2) * (nsk // 2)
            BIG = 10 ** 9
            items = []
            v_stage = {}

            def v_item(cg, blk, li):
                def _f():
                    if blk not in v_stage or v_stage[blk][1] != (cg,):
                        v_stage[blk] = (stage_v(blk), (cg,))
                    v_chunk(v_stage[blk][0], cg, li, blk * (sb_blk // P) + li)
                gi = blk * (sb_blk // P) + li
                dl = (4 * cg + 2) * 8 + gi // 4 - 2
                return (dl, 0, 900, _f)

            for cg in range(2):
                for blk in range(nsb):
                    for li in range(sb_blk // P):
                        items.append(v_item(cg, blk, li))

            for c in range(1, CC):
                for blk in range(nsb):
                    items.append((
                        max(0, (2 * c) * 8 + 2 * blk - 4), 0, 900,
                        lambda c=c, blk=blk: proj_qk_chunk(kst, wk, kT8, c, blk),
                    ))
            for jq in range(nsqb):
                for c in range(CC):
                    if jq == 0 and c == 0:
                        continue
                    items.append((
                        max(0, (8 * jq + 2 * c) * 8 - 6), 0, 900,
                        lambda c=c, jq=jq: proj_qk_chunk(
                            qst, wq, qT8, c, jq, bias=bq),
                    ))
            # output projection of row j: released once norm_fin of the
            # row's last head (step 8j+7) has landed (window 8j+10, group 3).
            for jo in range(nsqb):
                for sc in range(jo * (sqb // P), (jo + 1) * (sqb // P)):
                    for db in range(D // 512):
                        items.append((
                            BIG, (8 * jo + 10) * 8 + 5, 950,
                            lambda sc=sc, db=db: oproj_chunk(sc, db),
                        ))

            items.sort(key=lambda it: (it[0], it[1]))
            total_est = sum(it[2] for it in items)
            emitted_ns = 0.0

            def pump_fillers(slot, force_all=False):
                nonlocal emitted_ns
                target = (slot + 1) * total_est / total_slots
                while items:
                    k = None
                    for idx, it in enumerate(items):
                        if it[1] <= slot:
                            k = idx
                            break
                    if k is None:
                        return
                    dl = items[k][0]
                    if not (force_all or dl <= slot or emitted_ns < target):
                        return
                    it = items.pop(k)
                    it[3]()
                    emitted_ns += it[2]

            # ---------- pipelined attention ----------
            # window s: PE emits [PV(s-1, 2g..2g+1) | fillers | QK(s, g)]
            # per group g (blocking QK last so the in-order PE queue never
            # stalls), deferred bc/aT-mult of step s-2 after group 3, and
            # the DVE reciprocal chain of step s-1 at window end.
            E_cur = {}
            pv_ps = {}
            norm_state = {}

            def emit_qk_exp(sidx, g):
                j, h = steps[sidx]
                if g == 0:
                    E_t = ep.tile([P, nsk, sqb], dt16, tag="E")
                    E_cur[sidx] = E_t
                qkt = qkps.tile([P, 2, sqb], f32, tag="qk")
                qk_group(h, j, g, qkt)
                exp_group(E_cur[sidx], g, qkt)

            def emit_pv(sidx, g):
                j, h = steps[sidx]
                if g == 0:
                    pv = mmps.tile([P, sqb], f32, tag="mm")
                    pv_ps[sidx] = pv
                pv = pv_ps[sidx]
                E_t = E_cur[sidx]
                # halves are sequential accumulation groups in one bank:
                # groups 0-3 accumulate columns 0:256 (chunks 4g..4g+3),
                # groups 4-7 accumulate columns 256:512.
                hf, g4 = g // 4, g % 4
                for u in range(4):
                    i = 4 * g4 + u
                    nc.tensor.matmul(
                        pv[0:DH + 1, hf * 256:(hf + 1) * 256],
                        lhsT=vpad[:, i, h, :],
                        rhs=E_t[:, i, hf * 256:(hf + 1) * 256],
                        start=(i == 0),
                        stop=(i == nsk - 1),
                    )

            def norm_dve(sidx):
                pv = pv_ps.pop(sidx)
                E_cur.pop(sidx)
                zsb = rcp.tile([1, sqb], f32, tag="zsb")
                nc.vector.tensor_copy(out=zsb, in_=pv[DH:DH + 1, :])
                zf = rcp.tile([1, sqb], f32, tag="zf")
                nc.vector.reciprocal_approx_fast(out=zf, in_=zsb)
                aun = rcp.tile([DH, sqb], dt16, tag="aun")
                with nc.allow_low_precision(reason="fp16 attn out by design"):
                    nc.vector.tensor_copy(out=aun, in_=pv[0:DH, :])
                    rc = rcp.tile([1, sqb], dt16, tag="rc")
                    nc.vector.tensor_copy(out=rc, in_=zf)
                norm_state[sidx] = (aun, rc)

            def norm_fin(sidx):
                j, h = steps[sidx]
                aun, rc = norm_state.pop(sidx)
                js = slice(j * sqb, (j + 1) * sqb)
                p_i, hi = h // 2, h % 2
                bc = onps.tile([P, sqb], f32, tag="on")
                for hf in range(2):
                    nc.tensor.matmul(
                        bc[0:DH, hf * 256:(hf + 1) * 256], lhsT=ones_row,
                        rhs=rc[:, hf * 256:(hf + 1) * 256],
                        start=True, stop=True,
                    )
                with nc.allow_low_precision(reason="fp16 attn out by design"):
                    nc.vector.tensor_mul(
                        out=aT[64 * hi:64 * hi + DH, p_i, js],
                        in0=bc[0:DH, :],
                        in1=aun,
                    )

            # PV lags QK by TWO windows so E(s-2) is fully exp'd before any
            # PV matmul issues -- no PE wait embeds in PV slices; the QK(s)
            # <-> exp(s) ladder (qkps ring depth 2) is the only coupling.
            for sidx in range(nsteps + 2):
                for g in range(nsk // 2):
                    slot = sidx * (nsk // 2) + g
                    pump_fillers(slot)
                    if 2 <= sidx:
                        emit_pv(sidx - 2, g)
                    if g == 3 and sidx >= 3:
                        norm_fin(sidx - 3)
                    if sidx < nsteps:
                        emit_qk_exp(sidx, g)
                if sidx >= 2:
                    norm_dve(sidx - 2)
            norm_fin(nsteps - 1)
            pump_fillers(BIG, force_all=True)
    nc.compile()
    return nc


def make_in_maps(inputs, s=S):
    """Host-side sharding/layout prep. Returns per-core input dicts."""
    import ml_dtypes

    Q, K, V = inputs["Q"], inputs["K"], inputs["V"]
    Wq, Wk, Wv, Wo = inputs["Wq"], inputs["Wk"], inputs["Wv"], inputs["Wo"]
    bq = inputs["bq"]

    f16 = np.float16
    f8 = ml_dtypes.float8_e4m3
    qdt = f8 if PROJ_FP8 else f16
    QT = np.ascontiguousarray(np.asarray(Q).transpose(0, 2, 1)).astype(qdt)
    KT = np.ascontiguousarray(np.asarray(K).transpose(0, 2, 1)).astype(qdt)
    VT = np.ascontiguousarray(np.asarray(V).transpose(0, 2, 1)).astype(f16)

    per_hg = []
    for hg in range(2):
        sl = slice(hg * DL, (hg + 1) * DL)
        per_hg.append({
            "WQT": np.ascontiguousarray(np.asarray(Wq)[sl, :].T).astype(qdt),
            "WKT": np.ascontiguousarray(np.asarray(Wk)[sl, :].T).astype(qdt),
            "WVT": np.ascontiguousarray(np.asarray(Wv)[sl, :].T).astype(f16),
            "WOT": np.ascontiguousarray(np.asarray(Wo)[:, sl].T).astype(f16),
            "BQ": np.ascontiguousarray(
                np.asarray(bq)[sl].reshape(CC, P).T
            ).astype(np.float32),
        })

    in_maps = []
    for core in range(N_CORES):
        b, hg = core // 2, core % 2
        m = {"QT": QT[b], "KT": KT[b], "VT": VT[b]}
        m.update(per_hg[hg])
        in_maps.append(m)
    return in_maps


def assemble_output(inputs, results):
    Wo, bv, bo = inputs["Wo"], inputs["bv"], inputs["bo"]
    extra = (np.asarray(bv, np.float32) @ np.asarray(Wo, np.float32).T
             + np.asarray(bo, np.float32))
    out = np.zeros((B, S, D), np.float32)
    for core in range(N_CORES):
        out[core // 2] += results[core]["OUT"].astype(np.float32)
    out += extra[None, None, :]
    return out


_NC_CACHE = {}


def _get_nc(s=S):
    if s not in _NC_CACHE:
        _NC_CACHE[s] = build_bass(s)
    return _NC_CACHE[s]


def _run(inputs, trace=False):
    from concourse.bass_utils import run_bass_kernel_spmd

    nc = _get_nc()
    in_maps = make_in_maps(inputs)
    res = run_bass_kernel_spmd(nc, in_maps, list(range(N_CORES)), trace=trace)
    return assemble_output(inputs, res.results), res


def kernel(**inputs):
    return _run(inputs, trace=False)[0]


def kernel_traced(**inputs):
    return _run(inputs, trace=True)


# revision 41
# speedup vs baseline: 1.0214x; 1.0214x over previous
"""Trainium2 Bass kernel for nn_MultiHeadAttention_57337813402001.

B=4, S=2048, D=1024, H=16 heads (DH=64). 8 NeuronCores.

Sharding: core = (batch b, head-group hg); hg splits the 16 heads into two
groups of 8 (tensor parallel on the QKV projection output columns and the
output projection input rows), b is data parallel. Each core computes a
partial output projection for its 8 heads; the host sums the two partials
per batch and adds the (algebraically folded) bias terms.

Algebraic simplifications (exact in real arithmetic):
  - bk drops out of softmax (adds a per-query constant to scores).
  - bv commutes through the attention average: folded into a host-side bias
    row bv @ Wo^T added at the end.
  - softmax without max-subtraction: |scores|/sqrt(d) < ~0.7 here.

v2 dataflow (vs the 477us baseline):
  - Q/K path in fp8e4: the projections use MatmulPerfMode.DoubleRow (two
    d_in k-tiles per matmul -> half the instructions at the same ns/row =
    the real fp8 2x). QK^T runs plain fp8 in the natural [pair-dh, s]
    layout (DoubleRow cannot help a 64-deep contraction and its doubled
    weight load measured slower). Scores only feed exp(s/32), so fp8's ~4%
    score error is ~0.7% on probabilities; measured 3.8e-3 end to end.
  - V path and everything after exp stays fp16 (fp8 there costs ~4% output
    error).
  - Software-pipelined schedule: per (j-block, head) window, the PE emits
    [PV(step-1) | filler matmuls | QK(step)+exp(step)] per score group --
    the dependency-blocking QK last so the in-order PE queue never idles
    (the p-state model halves the PE clock for 3us after any gap). Filler
    work (V projection, later K/Q projection chunks, prior row's output
    projection) comes from a deadline + uniform-rate scheduler. The
    softmax-normalize reciprocal chain runs on DVE at window end, and its
    ones-broadcast matmul + aT multiply are deferred one window so the PE
    never waits on DVE.
  - PSUM: qk scores 2x[128,2,512] + mm 3x[128,512] + ones 1x[128,512]
    = 8 banks exactly.
"""

import os
import sys

import numpy as np

for _p in ("/opt/trn_rl_repo",):
    if _p not in sys.path and os.path.isdir(_p):
        sys.path.insert(0, _p)

B, S, D, H = 4, 2048, 1024, 16
DH = D // H          # 64
HL = H // 2          # 8 heads per core
DL = HL * DH         # 512 local hidden
P = 128
KC = D // P          # 8 d_in chunks
CC = DL // P         # 4 local d_out chunks
N_CORES = 8

QK_FP8 = True        # DoubleRow fp8 for QK^T scores
PROJ_FP8 = True      # DoubleRow fp8 for the Q/K projections (host fp8 in)


def build_bass(s=S):
    import concourse.bass as bass  # noqa: F401
    import concourse.mybir as mybir
    from concourse import bacc
    from concourse.tile import TileContext

    dt16 = mybir.dt.float16
    f8 = mybir.dt.float8e4
    f32 = mybir.dt.float32
    AF = mybir.ActivationFunctionType
    DR = mybir.MatmulPerfMode.DoubleRow

    nsk = s // P                 # sk chunks (16)
    sqb = min(512, s)            # sq block
    nsqb = s // sqb              # 4 j rows
    sb_blk = min(512, s)
    nsb = s // sb_blk            # 4 projection s blocks
    qdt = f8 if PROJ_FP8 else dt16

    nc = bacc.Bacc()
    QT = nc.declare_dram_parameter("QT", [D, s], qdt, isOutput=False)
    KT = nc.declare_dram_parameter("KT", [D, s], qdt, isOutput=False)
    VT = nc.declare_dram_parameter("VT", [D, s], dt16, isOutput=False)
    WQT = nc.declare_dram_parameter("WQT", [D, DL], qdt, isOutput=False)
    WKT = nc.declare_dram_parameter("WKT", [D, DL], qdt, isOutput=False)
    WVT = nc.declare_dram_parameter("WVT", [D, DL], dt16, isOutput=False)
    WOT = nc.declare_dram_parameter("WOT", [DL, D], dt16, isOutput=False)
    BQ = nc.declare_dram_parameter("BQ", [P, CC], f32, isOutput=False)
    OUT = nc.declare_dram_parameter("OUT", [s, D], dt16, isOutput=True)

    with TileContext(nc) as tc:
        with (
            tc.tile_pool(name="w", bufs=1) as wp,
            tc.tile_pool(name="stq", bufs=1) as stq,
            tc.tile_pool(name="stk", bufs=1) as stk,
            tc.tile_pool(name="stv", bufs=2) as stv,
            tc.tile_pool(name="qkv", bufs=1) as qkvp,
            tc.tile_pool(name="E", bufs=3) as ep,
            tc.tile_pool(name="rc", bufs=2) as rcp,
            tc.tile_pool(name="ost", bufs=3) as ostp,
            tc.tile_pool(name="qkps", bufs=2, space="PSUM") as qkps,
            tc.tile_pool(name="mmps", bufs=3, space="PSUM") as mmps,
            tc.tile_pool(name="onps", bufs=1, space="PSUM") as onps,
        ):
            # --- weights / constants ---
            wq = wp.tile([P, KC, DL], qdt, tag="wq")
            wk = wp.tile([P, KC, DL], qdt, tag="wk")
            wv = wp.tile([P, KC, DL], dt16, tag="wv")
            wo = wp.tile([P, CC, D], dt16, tag="wo")
            bq = wp.tile([P, CC], f32, tag="bq")
            ones_row = wp.tile([1, DH], dt16, tag="ones")
            qst = stq.tile([P, KC, s], qdt, tag="qst")
            kst = stk.tile([P, KC, s], qdt, tag="kst")

            # DMA issue order feeds the critical path: k projection of
            # s-block b needs only wk + kst block b; q j0 needs wq + qst
            # block 0. Everything else follows.
            def stage_blk(dst, src, blk):
                nc.sync.dma_start(
                    dst[:, :, blk * sb_blk:(blk + 1) * sb_blk],
                    src[:, blk * sb_blk:(blk + 1) * sb_blk].rearrange(
                        "(kc p) ss -> p kc ss", p=P
                    ),
                )

            nc.sync.dma_start(wk, WKT[:].rearrange("(kc p) m -> p kc m", p=P))
            stage_blk(kst, KT, 0)
            nc.sync.dma_start(wq, WQT[:].rearrange("(kc p) m -> p kc m", p=P))
            nc.sync.dma_start(bq, BQ[:])
            stage_blk(qst, QT, 0)
            for blk in range(1, nsb):
                stage_blk(kst, KT, blk)
            nc.sync.dma_start(wv, WVT[:].rearrange("(kc p) m -> p kc m", p=P))
            for blk in range(1, nsb):
                stage_blk(qst, QT, blk)
            nc.sync.dma_start(wo, WOT[:].rearrange("(cc p) m -> p cc m", p=P))
            nc.vector.memset(ones_row, 1.0)

            # q/k fp8 in natural projection layout: [128, pair, s], partition
            # = 64*hi + dh for head 2*pair + hi. QK^T runs plain fp8 (K=64):
            # DoubleRow can't help a 64-deep contraction, and its doubled
            # weight load measured ~50ns/matmul slower.
            qT8 = qkvp.tile([P, CC, s], f8, tag="qT8")
            kT8 = qkvp.tile([P, CC, s], f8, tag="kT8")
            vpad = qkvp.tile([P, nsk, HL, DH + 1], dt16, tag="vpad")
            aT = qkvp.tile([P, CC, s], dt16, tag="aT")
            nc.vector.memset(vpad[:, :, :, DH], 1.0)

            scale = 1.0 / np.sqrt(np.float32(D)).item()

            # ---------- emission helpers ----------
            def proj_qk_chunk(xst, w, dst8, c, blk, bias=None):
                """One [128,512] chunk of a q/k projection (fp8 DoubleRow,
                2 d_in k-tiles per matmul), cast straight into dst8[:, c]."""
                ps = mmps.tile([P, sb_blk], f32, tag="mm")
                if PROJ_FP8:
                    for k2 in range(KC // 2):
                        nc.tensor.matmul(
                            ps,
                            lhsT=w[:, 2 * k2:2 * k2 + 2, c * P:(c + 1) * P],
                            rhs=xst[:, 2 * k2:2 * k2 + 2,
                                    blk * sb_blk:(blk + 1) * sb_blk],
                            start=(k2 == 0),
                            stop=(k2 == KC // 2 - 1),
                            perf_mode=DR,
                        )
                else:
                    for k in range(KC):
                        nc.tensor.matmul(
                            ps,
                            lhsT=w[:, k, c * P:(c + 1) * P],
                            rhs=xst[:, k, blk * sb_blk:(blk + 1) * sb_blk],
                            start=(k == 0),
                            stop=(k == KC - 1),
                        )
                dsl = dst8[:, c, blk * sb_blk:(blk + 1) * sb_blk]
                with nc.allow_low_precision(reason="fp8 q/k by design"):
                    if bias is not None:
                        nc.vector.tensor_scalar_add(
                            out=dsl, in0=ps, scalar1=bias[:, c:c + 1],
                        )
                    else:
                        nc.vector.tensor_copy(out=dsl, in_=ps)

            def v_chunk(xv, cg, li, gi):
                """V projection for pair-group cg (pairs 2cg,2cg+1), local
                chunk li of the staged block = global sk chunk gi; N=256."""
                ps = mmps.tile([P, 256], f32, tag="mm")
                for k in range(KC):
                    nc.tensor.matmul(
                        ps,
                        lhsT=xv[:, k, li * P:(li + 1) * P],
                        rhs=wv[:, k, cg * 256:(cg + 1) * 256],
                        start=(k == 0),
                        stop=(k == KC - 1),
                    )
                with nc.allow_low_precision(reason="fp16 v by design"):
                    nc.vector.tensor_copy(
                        out=vpad[:, gi, 4 * cg:4 * cg + 4, 0:DH],
                        in_=ps.rearrange("p (h d) -> p h d", d=DH),
                    )

            def qk_group(h, j, g, qkt):
                """Scores^T chunks 2g,2g+1 for head h, q block j -> qkt."""
                p_i, hi = h // 2, h % 2
                po = 64 * hi
                js = slice(j * sqb, (j + 1) * sqb)
                for u in range(2):
                    i = 2 * g + u
                    nc.tensor.matmul(
                        qkt[:, u, :],
                        lhsT=kT8[po:po + DH, p_i, i * P:(i + 1) * P],
                        rhs=qT8[po:po + DH, p_i, js],
                        start=True, stop=True,
                    )

            def exp_group(E_t, g, qkt):
                with nc.allow_low_precision(reason="fp16 probs by design"):
                    nc.scalar.activation(
                        out=E_t[:, 2 * g:2 * g + 2, :], in_=qkt,
                        func=AF.Exp, scale=scale,
                    )

            # ---------- filler queue (PE work interleaved into the
            # attention pipeline; ~each item <= ~1us of PE time) ----------
            # V staging tiles cycle; stage block DMAs are emitted lazily
            # right before first use (SP queue is in-order).
            def stage_v(blk):
                xv = stv.tile([P, KC, sb_blk], dt16, tag="stv")
                nc.sync.dma_start(
                    xv,
                    VT[:, blk * sb_blk:(blk + 1) * sb_blk].rearrange(
                        "(kc p) ss -> p kc ss", p=P
                    ),
                )
                return xv

            def oproj_chunk(sc, db):
                ps = mmps.tile([P, 512], f32, tag="mm")
                for c in range(CC):
                    nc.tensor.matmul(
                        ps,
                        lhsT=aT[:, c, sc * P:(sc + 1) * P],
                        rhs=wo[:, c, db * 512:(db + 1) * 512],
                        start=(c == 0),
                        stop=(c == CC - 1),
                    )
                ot = ostp.tile([P, 512], dt16, tag="ost")
                with nc.allow_low_precision(reason="fp16 partial"):
                    nc.vector.tensor_copy(out=ot, in_=ps)
                nc.sync.dma_start(
                    OUT[sc * P:(sc + 1) * P, db * 512:(db + 1) * 512], ot
                )

            # ---------- preamble: only k c0 blk0 + q j0 c0 gate QK(0,0);
            # k c0 blk1-3 become tight-deadline fillers (QK(0, g) reads
            # s-chunks 2g,2g+1 = block g//2) so the first scores issue as
            # soon as the first two staging DMAs land.
            proj_qk_chunk(kst, wk, kT8, 0, 0)
            proj_qk_chunk(qst, wq, qT8, 0, 0, bias=bq)

            # ---------- filler scheduler ----------
            # Items = (deadline_slot, release_slot, est_pe_ns, fn), emitted
            # into group slots at a uniform PE-time rate with deadline
            # forcing, so the in-order PE queue always has ready work (the
            # p-state model halves the PE clock for 3us after any idle gap).
            # slot = window * 8 + group; windows = nsteps + 1.
            steps = [(j, h) for j in range(nsqb) for h in range(HL)]
            nsteps = len(steps)
            total_slots = (nsteps + 2) * (nsk // 2)
            BIG = 10 ** 9
            items = []
            v_stage = {}

            def v_item(cg, blk, li):
                def _f():
                    if blk not in v_stage or v_stage[blk][1] != (cg,):
                        v_stage[blk] = (stage_v(blk), (cg,))
                    v_chunk(v_stage[blk][0], cg, li, blk * (sb_blk // P) + li)
                gi = blk * (sb_blk // P) + li
                dl = (4 * cg + 2) * 8 + gi // 2 - 2
                return (dl, 0, 900, _f)

            for cg in range(2):
                for blk in range(nsb):
                    for li in range(sb_blk // P):
                        items.append(v_item(cg, blk, li))

            for c in range(CC):
                for blk in range(nsb):
                    if c == 0 and blk == 0:
                        continue
                    # deadline: QK(step 2c, group 2*blk) reads this block;
                    # release keeps early c0 blocks from popping before the
                    # first QK groups (their staging DMAs are still inflight).
                    items.append((
                        max(1, (2 * c) * 8 + 2 * blk - 4),
                        max(0, 2 * blk - 1) if c == 0 else 0, 900,
                        lambda c=c, blk=blk: proj_qk_chunk(kst, wk, kT8, c, blk),
                    ))
            for jq in range(nsqb):
                for c in range(CC):
                    if jq == 0 and c == 0:
                        continue
                    items.append((
                        max(0, (8 * jq + 2 * c) * 8 - 6), 0, 900,
                        lambda c=c, jq=jq: proj_qk_chunk(
                            qst, wq, qT8, c, jq, bias=bq),
                    ))
            # output projection of row j: released once norm_fin of the
            # row's last head (step 8j+7) has landed (window 8j+10, group 3).
            for jo in range(nsqb):
                for sc in range(jo * (sqb // P), (jo + 1) * (sqb // P)):
                    for db in range(D // 512):
                        items.append((
                            BIG, (8 * jo + 10) * 8 + 5, 950,
                            lambda sc=sc, db=db: oproj_chunk(sc, db),
                        ))

            items.sort(key=lambda it: (it[0], it[1]))
            total_est = sum(it[2] for it in items)
            emitted_ns = 0.0

            def pump_fillers(slot, force_all=False):
                nonlocal emitted_ns
                target = (slot + 1) * total_est / total_slots
                while items:
                    k = None
                    for idx, it in enumerate(items):
                        if it[1] <= slot:
                            k = idx
                            break
                    if k is None:
                        return
                    dl = items[k][0]
                    if not (force_all or dl <= slot or emitted_ns < target):
                        return
                    it = items.pop(k)
                    it[3]()
                    emitted_ns += it[2]

            # ---------- pipelined attention ----------
            # window s: PE emits [PV(s-1, 2g..2g+1) | fillers | QK(s, g)]
            # per group g (blocking QK last so the in-order PE queue never
            # stalls), deferred bc/aT-mult of step s-2 after group 3, and
            # the DVE reciprocal chain of step s-1 at window end.
            E_cur = {}
            pv_ps = {}
            norm_state = {}

            def emit_qk_exp(sidx, g):
                j, h = steps[sidx]
                if g == 0:
                    E_t = ep.tile([P, nsk, sqb], dt16, tag="E")
                    E_cur[sidx] = E_t
                qkt = qkps.tile([P, 2, sqb], f32, tag="qk")
                qk_group(h, j, g, qkt)
                exp_group(E_cur[sidx], g, qkt)

            def emit_pv(sidx, g):
                j, h = steps[sidx]
                if g == 0:
                    pv = mmps.tile([P, sqb], f32, tag="mm")
                    pv_ps[sidx] = pv
                pv = pv_ps[sidx]
                E_t = E_cur[sidx]
                for u in range(2):
                    i = 2 * g + u
                    nc.tensor.matmul(
                        pv[0:DH + 1, :],
                        lhsT=vpad[:, i, h, :],
                        rhs=E_t[:, i, :],
                        start=(i == 0),
                        stop=(i == nsk - 1),
                    )

            def norm_dve(sidx):
                pv = pv_ps.pop(sidx)
                E_cur.pop(sidx)
                zsb = rcp.tile([1, sqb], f32, tag="zsb")
                nc.vector.tensor_copy(out=zsb, in_=pv[DH:DH + 1, :])
                zf = rcp.tile([1, sqb], f32, tag="zf")
                nc.vector.reciprocal_approx_fast(out=zf, in_=zsb)
                aun = rcp.tile([DH, sqb], dt16, tag="aun")
                with nc.allow_low_precision(reason="fp16 attn out by design"):
                    nc.vector.tensor_copy(out=aun, in_=pv[0:DH, :])
                    rc = rcp.tile([1, sqb], dt16, tag="rc")
                    nc.vector.tensor_copy(out=rc, in_=zf)
                norm_state[sidx] = (aun, rc)

            def norm_fin(sidx):
                j, h = steps[sidx]
                aun, rc = norm_state.pop(sidx)
                js = slice(j * sqb, (j + 1) * sqb)
                p_i, hi = h // 2, h % 2
                bc = onps.tile([P, sqb], f32, tag="on")
                nc.tensor.matmul(
                    bc[0:DH, :], lhsT=ones_row, rhs=rc, start=True, stop=True,
                )
                with nc.allow_low_precision(reason="fp16 attn out by design"):
                    nc.vector.tensor_mul(
                        out=aT[64 * hi:64 * hi + DH, p_i, js],
                        in0=bc[0:DH, :],
                        in1=aun,
                    )

            # PV lags QK by TWO windows so E(s-2) is fully exp'd before any
            # PV matmul issues -- no PE wait embeds in PV slices; the QK(s)
            # <-> exp(s) ladder (qkps ring depth 2) is the only coupling.
            for sidx in range(nsteps + 2):
                for g in range(nsk // 2):
                    slot = sidx * (nsk // 2) + g
                    if 2 <= sidx:
                        emit_pv(sidx - 2, g)
                    pump_fillers(slot)
                    if g == 3 and sidx >= 3:
                        norm_fin(sidx - 3)
                    if sidx < nsteps:
                        emit_qk_exp(sidx, g)
                if sidx >= 2:
                    norm_dve(sidx - 2)
            norm_fin(nsteps - 1)
            pump_fillers(BIG, force_all=True)
    nc.compile()
    return nc


def make_in_maps(inputs, s=S):
    """Host-side sharding/layout prep. Returns per-core input dicts."""
    import ml_dtypes

    Q, K, V = inputs["Q"], inputs["K"], inputs["V"]
    Wq, Wk, Wv, Wo = inputs["Wq"], inputs["Wk"], inputs["Wv"], inputs["Wo"]
    bq = inputs["bq"]

    f16 = np.float16
    f8 = ml_dtypes.float8_e4m3
    qdt = f8 if PROJ_FP8 else f16
    QT = np.ascontiguousarray(np.asarray(Q).transpose(0, 2, 1)).astype(qdt)
    KT = np.ascontiguousarray(np.asarray(K).transpose(0, 2, 1)).astype(qdt)
    VT = np.ascontiguousarray(np.asarray(V).transpose(0, 2, 1)).astype(f16)

    per_hg = []
    for hg in range(2):
        sl = slice(hg * DL, (hg + 1) * DL)
        per_hg.append({
            "WQT": np.ascontiguousarray(np.asarray(Wq)[sl, :].T).astype(qdt),
            "WKT": np.ascontiguousarray(np.asarray(Wk)[sl, :].T).astype(qdt),
            "WVT": np.ascontiguousarray(np.asarray(Wv)[sl, :].T).astype(f16),
            "WOT": np.ascontiguousarray(np.asarray(Wo)[:, sl].T).astype(f16),
            "BQ": np.ascontiguousarray(
                np.asarray(bq)[sl].reshape(CC, P).T
            ).astype(np.float32),
        })

    in_maps = []
    for core in range(N_CORES):
        b, hg = core // 2, core % 2
        m = {"QT": QT[b], "KT": KT[b], "VT": VT[b]}
        m.update(per_hg[hg])
        in_maps.append(m)
    return in_maps


def assemble_output(inputs, results):
    Wo, bv, bo = inputs["Wo"], inputs["bv"], inputs["bo"]
    extra = (np.asarray(bv, np.float32) @ np.asarray(Wo, np.float32).T
             + np.asarray(bo, np.float32))
    out = np.zeros((B, S, D), np.float32)
    for core in range(N_CORES):
        out[core // 2] += results[core]["OUT"].astype(np.float32)
    out += extra[None, None, :]
    return out


_NC_CACHE = {}


def _get_nc(s=S):
    if s not in _NC_CACHE:
        _NC_CACHE[s] = build_bass(s)
    return _NC_CACHE[s]


def _run(inputs, trace=False):
    from concourse.bass_utils import run_bass_kernel_spmd

    nc = _get_nc()
    in_maps = make_in_maps(inputs)
    res = run_bass_kernel_spmd(nc, in_maps, list(range(N_CORES)), trace=trace)
    return assemble_output(inputs, res.results), res


def kernel(**inputs):
    return _run(inputs, trace=False)[0]


def kernel_traced(**inputs):
    return _run(inputs, trace=True)
